# revision 9
# baseline (speedup 1.0000x reference)
"""Segment mean-pool (ContextEncoder) Bass kernel for Trainium2, 8 NeuronCores.

Problem: hidden_states [16, 4096, 1024] f32, output_mask [16, 4096] int
(non-decreasing segment ids per row in [0, 512), -1 = dropped token).
Output [16*512, 1024] f32: mean of tokens sharing (batch, segment id),
zeros for empty segments.

Strategy: data-parallel over batch, 2 rows per core. Per 128-token K-tile,
build a one-hot [tokens x 512 segments] matrix on the vector engine
(iota vs per-partition segment id, is_equal), then accumulate
one_hot.T @ x on the tensor engine (fp32r) into PSUM, one [128 seg x 1024]
region per 128-segment M-tile. Because ids are sorted, each K-tile only
touches 1-2 M-tiles; the (k -> M-tiles) map is computed on the host from
the actual masks (the program is compiled per input batch) so the matmul
count stays near the minimum while remaining exact for any mask content.
Mean = PSUM * (1/count) on the way out, with counts from a host bincount.
"""

import numpy as np

import concourse.bass as bass  # noqa: F401  (registers bass_rust)
import concourse.mybir as mybir
import concourse.tile as tile
from concourse import bacc
from concourse.bass_utils import run_bass_kernel_spmd

B, S, H = 16, 4096, 1024
NSEG = 512
NCORES = 8
RPC = B // NCORES          # rows (batch examples) per core
P = 128                    # partitions
KT = S // P                # 32 K-tiles of 128 tokens
MT = NSEG // P             # 4 M-tiles of 128 segments
NH = H // 512              # matmul free-dim chunks (PSUM bank = 512 f32)

F32 = mybir.dt.float32
F32R = mybir.dt.float32r   # full-rate fp32 matmul mode on TRN2
F16 = mybir.dt.float16

# "fp16": half input DMA traffic (memory-bound win), ~2-4e-4 rel err.
# "fp32r": full fp32 traffic, ~1.6e-4 rel err.
PRECISION = "fp16"

# Number of SBUF buffers for data tiles (DMA prefetch depth)
DATA_BUFS = 8
OH_BUFS = 6
OSB_BUFS = 4


def _build_program(klists, loop_n=1, precision=None):
    """klists[r][m] -> sorted list of K-tile indices whose token ids (in any
    row assigned to program slot r) overlap segment M-tile m. Must be
    non-empty for every (r, m).

    loop_n > 1 wraps the body in an in-NEFF repeat loop (timing only)."""
    precision = precision or PRECISION
    ddt = F16 if precision == "fp16" else F32R
    nc = bacc.Bacc("TRN2", target_bir_lowering=False, debug=False)
    x = nc.dram_tensor("x", [RPC, S, H], ddt, kind="ExternalInput")
    maskp = nc.dram_tensor("maskp", [RPC, P, KT], F32, kind="ExternalInput")
    invc = nc.dram_tensor("invc", [RPC, P, MT], F32, kind="ExternalInput")
    out = nc.dram_tensor("out", [RPC, NSEG, H], F32, kind="ExternalOutput")

    with tile.TileContext(nc) as tc:
        with tc.tile_pool(name="const", bufs=1) as cpool, \
             tc.tile_pool(name="data", bufs=DATA_BUFS) as dpool, \
             tc.tile_pool(name="oh", bufs=OH_BUFS) as opool, \
             tc.tile_pool(name="osb", bufs=OSB_BUFS) as spool, \
             tc.tile_pool(name="ps", bufs=MT, space="PSUM") as pspool:
            iota_t = cpool.tile([P, NSEG], F32, tag="iota")
            nc.gpsimd.iota(iota_t[:], [[1, NSEG]], channel_multiplier=0,
                           allow_small_or_imprecise_dtypes=True)
            body = _make_body(nc, klists, x, maskp, invc, out, iota_t,
                              cpool, dpool, opool, spool, pspool, ddt)
            if loop_n > 1:
                with tc.For_i(0, loop_n, 1):
                    body()
            else:
                body()
    nc.compile()
    return nc


def _make_body(nc, klists, x, maskp, invc, out, iota_t,
               cpool, dpool, opool, spool, pspool, ddt):
    def body():
        for r in range(RPC):
                mask_sb = cpool.tile([P, KT], F32, tag=f"mask{r}")
                nc.sync.dma_start(out=mask_sb[:], in_=maskp[r])
                invc_sb = cpool.tile([P, MT], F32, tag=f"invc{r}")
                nc.sync.dma_start(out=invc_sb[:], in_=invc[r])

                k_to_ms = {}
                for m in range(MT):
                    for k in klists[r][m]:
                        k_to_ms.setdefault(k, []).append(m)
                firsts = {m: klists[r][m][0] for m in range(MT)}
                lasts = {m: klists[r][m][-1] for m in range(MT)}

                psum = [pspool.tile([P, H], F32, tag="ps", name=f"psum_r{r}m{m}")
                        for m in range(MT)]

                for k in sorted(k_to_ms):
                    data_t = dpool.tile([P, H], ddt, tag="data")
                    nc.sync.dma_start(out=data_t[:], in_=x[r, k * P:(k + 1) * P, :])
                    oh = opool.tile([P, NSEG], ddt, tag="oh")
                    nc.vector.tensor_scalar(
                        out=oh[:], in0=iota_t[:], scalar1=mask_sb[:, k:k + 1],
                        scalar2=None, op0=mybir.AluOpType.is_equal)
                    for m in k_to_ms[k]:
                        for n in range(NH):
                            nc.tensor.matmul(
                                out=psum[m][:, n * 512:(n + 1) * 512],
                                lhsT=oh[:, m * P:(m + 1) * P],
                                rhs=data_t[:, n * 512:(n + 1) * 512],
                                start=(k == firsts[m]), stop=(k == lasts[m]))

                for m in range(MT):
                    osb = spool.tile([P, H], F32, tag="osb")
                    nc.vector.tensor_scalar_mul(osb[:], psum[m][:], invc_sb[:, m:m + 1])
                    nc.sync.dma_start(out=out[r, m * P:(m + 1) * P, :], in_=osb[:])
    return body


def _prep(hidden_states, output_mask, precision=None):
    precision = precision or PRECISION
    np_ddt = np.float16 if precision == "fp16" else np.float32
    hs = np.ascontiguousarray(np.asarray(hidden_states).astype(np_ddt))
    assert hs.shape == (B, S, H), hs.shape
    mask = np.asarray(output_mask).astype(np.int64)
    assert mask.shape == (B, S), mask.shape

    valid = mask >= 0
    # per-(row, K-tile) id range over valid tokens
    m3 = mask.reshape(B, KT, P)
    v3 = valid.reshape(B, KT, P)
    lo = np.where(v3, m3, np.iinfo(np.int64).max).min(axis=2)  # [B, KT]
    hi = np.where(v3, m3, -1).max(axis=2)                      # [B, KT]

    klists = []
    for r in range(RPC):
        rows = [c * RPC + r for c in range(NCORES)]
        per_m = []
        for m in range(MT):
            ks = [k for k in range(KT)
                  if any(lo[b, k] <= m * P + P - 1 and hi[b, k] >= m * P
                         for b in rows)]
            per_m.append(ks if ks else [0])
        klists.append(per_m)

    counts = np.zeros((B, NSEG), np.int64)
    for b in range(B):
        ids = mask[b][valid[b]]
        ids = ids[ids < NSEG]
        counts[b] = np.bincount(ids, minlength=NSEG)
    invc = (1.0 / np.maximum(counts, 1)).astype(np.float32)

    maskp = np.ascontiguousarray(
        mask.astype(np.float32).reshape(B, KT, P).transpose(0, 2, 1))
    invcp = np.ascontiguousarray(
        invc.reshape(B, MT, P).transpose(0, 2, 1))

    in_maps = [{
        "x": hs[c * RPC:(c + 1) * RPC],
        "maskp": maskp[c * RPC:(c + 1) * RPC],
        "invc": invcp[c * RPC:(c + 1) * RPC],
    } for c in range(NCORES)]
    return klists, in_maps


_PROGRAM_CACHE = {}


def _get_program(klists):
    key = (PRECISION,
           tuple(tuple(tuple(ks) for ks in per_m) for per_m in klists))
    if key not in _PROGRAM_CACHE:
        _PROGRAM_CACHE[key] = _build_program(klists)
    return _PROGRAM_CACHE[key]


def kernel(hidden_states, output_mask):
    klists, in_maps = _prep(hidden_states, output_mask)
    nc = _get_program(klists)
    res = run_bass_kernel_spmd(nc, in_maps, core_ids=list(range(NCORES)))
    full = np.concatenate(
        [res.results[c]["out"].reshape(RPC * NSEG, H) for c in range(NCORES)],
        axis=0)
    return full


if __name__ == "__main__":
    rng = np.random.default_rng(0)
    hs = rng.standard_normal((B, S, H)).astype(np.float32)
    mask = np.sort(rng.integers(0, NSEG, size=(B, S)), axis=-1).astype(np.int32)
    out = kernel(hidden_states=hs, output_mask=mask)
    print(out.shape, out.dtype)


# revision 19
# speedup vs baseline: 1.6192x; 1.6192x over previous
"""Segment mean-pool (ContextEncoder) Bass kernel for Trainium2, 8 NeuronCores.

Problem: hidden_states [16, 4096, 1024] f32, output_mask [16, 4096] int
(non-decreasing segment ids per row in [0, 512), -1 = dropped token).
Output [16*512, 1024] f32: mean of tokens sharing (batch, segment id),
zeros for empty segments.

Strategy: data-parallel over batch, 2 rows per core. Per 128-token K-tile,
build a one-hot [tokens x segments] matrix on the vector engine (iota vs
per-partition segment id, is_equal), then accumulate one_hot.T @ x on the
tensor engine (fp16 operands, fp32 PSUM accumulate), one [128 seg x 1024]
PSUM region per 128-segment M-tile. Because ids are sorted, each K-tile
only touches 1-2 M-tiles; the (k -> M-tiles) map is computed on the host
from the actual masks (the program is compiled per input batch) so the
matmul count stays near the minimum while remaining exact for any mask
content. Mean = PSUM * (1/count) on an ACT-engine drain (counts from a
host bincount), written back as fp16 and upcast on the host.

Memory-system notes (measured on HW via in-NEFF loop + paired timing):
input reads stream at ~440 GB/s/core when grouped 2 K-tiles per DMA on a
dedicated HWDGE ring; HBM writes only reach ~100-140 GB/s and do not
overlap reads, so outputs are fp16 and coalesced into one DMA per row.
"""

import numpy as np

import concourse.bass as bass  # noqa: F401  (registers bass_rust)
import concourse.mybir as mybir
import concourse.tile as tile
from concourse import bacc
from concourse.bass_utils import run_bass_kernel_spmd

B, S, H = 16, 4096, 1024
NSEG = 512
NCORES = 8
RPC = B // NCORES          # rows (batch examples) per core
P = 128                    # partitions
KT = S // P                # 32 K-tiles of 128 tokens
MT = NSEG // P             # 4 M-tiles of 128 segments
NH = H // 512              # matmul free-dim chunks (PSUM bank = 512 f32)

F32 = mybir.dt.float32
F32R = mybir.dt.float32r   # full-rate fp32 matmul mode on TRN2
F16 = mybir.dt.float16

# "fp16": half input DMA traffic (memory-bound win), ~2-4e-4 rel err.
# "fp32r": full fp32 traffic, ~1.6e-4 rel err.
PRECISION = "fp16"

# Number of SBUF buffers for data tiles (DMA prefetch depth)
DATA_BUFS = 16
OH_BUFS = 32
OSB_BUFS = 2
KPG = 2            # k-tiles per input DMA (bigger DMAs amortize per-DMA cost)
IN_RING = "sync"   # HWDGE ring for input DMAs (dedicated: avoids head-of-line
OUT_RING = "scalar"  # blocking of inputs behind output DMAs waiting on drains)
MODE = "full"      # "full" | "dma_only" | "compute_only" | "no_out" | "out_only"
OUT_CHUNK = 1      # m-tiles per output DMA (out_only diagnostics)
OUT_ALT = False    # alternate output DMAs across both rings
OUT_FP16 = True    # device writes fp16 outputs (half the slow HBM write
                   # traffic); host upcasts to f32 after gather
OUT_COMBINE = True  # one output DMA per row instead of one per m-tile


def _build_program(klists, loop_n=1, precision=None):
    """klists[r][m] -> sorted list of K-tile indices whose token ids (in any
    row assigned to program slot r) overlap segment M-tile m. Must be
    non-empty for every (r, m).

    loop_n > 1 wraps the body in an in-NEFF repeat loop (timing only)."""
    precision = precision or PRECISION
    ddt = F16 if precision == "fp16" else F32R
    nc = bacc.Bacc("TRN2", target_bir_lowering=False, debug=False)
    x = nc.dram_tensor("x", [RPC, S, H], ddt, kind="ExternalInput")
    maskp = nc.dram_tensor("maskp", [RPC, P, KT], F32, kind="ExternalInput")
    invc = nc.dram_tensor("invc", [RPC, P, MT], F32, kind="ExternalInput")
    odt = F16 if OUT_FP16 else F32
    out = nc.dram_tensor("out", [RPC, NSEG, H], odt, kind="ExternalOutput")

    with tile.TileContext(nc) as tc:
        with tc.tile_pool(name="const", bufs=1) as cpool, \
             tc.tile_pool(name="data", bufs=DATA_BUFS) as dpool, \
             tc.tile_pool(name="oh", bufs=OH_BUFS) as opool, \
             tc.tile_pool(name="osb", bufs=OSB_BUFS) as spool, \
             tc.tile_pool(name="ps", bufs=MT, space="PSUM") as pspool:
            iota_t = cpool.tile([P, NSEG], F32, tag="iota")
            nc.gpsimd.iota(iota_t[:], [[1, NSEG]], channel_multiplier=0,
                           allow_small_or_imprecise_dtypes=True)
            body = _make_body(nc, klists, x, maskp, invc, out, iota_t,
                              cpool, dpool, opool, spool, pspool, ddt, odt)
            if loop_n > 1:
                with tc.For_i(0, loop_n, 1):
                    body()
            else:
                body()
    nc.compile()
    return nc


def _make_body(nc, klists, x, maskp, invc, out, iota_t,
               cpool, dpool, opool, spool, pspool, ddt, odt):
    in_eng = getattr(nc, IN_RING)
    out_eng = getattr(nc, OUT_RING)

    def body():
        for r in range(RPC):
            mask_sb = cpool.tile([P, KT], F32, tag=f"mask{r}")
            nc.sync.dma_start(out=mask_sb[:], in_=maskp[r])
            invc_sb = cpool.tile([P, MT], F32, tag=f"invc{r}")
            nc.sync.dma_start(out=invc_sb[:], in_=invc[r])

            k_to_ms = {}
            for m in range(MT):
                for k in klists[r][m]:
                    k_to_ms.setdefault(k, []).append(m)
            firsts = {m: klists[r][m][0] for m in range(MT)}
            lasts = {m: klists[r][m][-1] for m in range(MT)}

            psum = [pspool.tile([P, H], F32, tag="ps", name=f"psum_r{r}m{m}")
                    for m in range(MT)]

            if MODE == "out_only":
                # OUT_CHUNK m-tiles per write DMA; alternate rings if OUT_ALT
                osb0 = spool.tile([P, MT, H], odt, tag="osb", name=f"osb_{r}")
                nc.vector.memset(osb0[:], 0.25)
                orv = out[r, :, :].rearrange("(m p) h -> p m h", p=P)
                for i, m in enumerate(range(0, MT, OUT_CHUNK)):
                    eng = (in_eng if (OUT_ALT and i % 2) else out_eng)
                    eng.dma_start(out=orv[:, m:m + OUT_CHUNK, :],
                                  in_=osb0[:, m:m + OUT_CHUNK, :])
                continue
            # x[r] tokens t = k*P + p; view as [p, k, h] for grouped loads
            xr = x[r, :, :].rearrange("(k p) h -> p k h", p=P)
            cdata = None
            for k0 in range(0, KT, KPG):
                group = [k for k in range(k0, min(k0 + KPG, KT)) if k in k_to_ms]
                if not group:
                    continue
                g = min(k0 + KPG, KT) - k0
                if MODE == "compute_only":
                    if cdata is None:
                        cdata = dpool.tile([P, KPG, H], ddt, tag="data",
                                           name=f"data_{r}")
                        in_eng.dma_start(out=cdata[:], in_=xr[:, 0:KPG, :])
                    data_t = cdata
                else:
                    data_t = dpool.tile([P, g, H], ddt, tag="data",
                                        name=f"data_{r}_{k0}")
                    in_eng.dma_start(out=data_t[:], in_=xr[:, k0:k0 + g, :])
                if MODE == "dma_only":
                    continue
                for k in group:
                    ms = k_to_ms[k]
                    m0, span = ms[0], ms[-1] - ms[0] + 1
                    oh = opool.tile([P, span * P], ddt, tag="oh",
                                    name=f"oh_{r}_{k}")
                    nc.vector.tensor_scalar(
                        out=oh[:], in0=iota_t[:, m0 * P:(m0 + span) * P],
                        scalar1=mask_sb[:, k:k + 1],
                        scalar2=None, op0=mybir.AluOpType.is_equal)
                    for m in ms:
                        for n in range(NH):
                            nc.tensor.matmul(
                                out=psum[m][:, n * 512:(n + 1) * 512],
                                lhsT=oh[:, (m - m0) * P:(m - m0 + 1) * P],
                                rhs=data_t[:, k - k0, n * 512:(n + 1) * 512],
                                start=(k == firsts[m]), stop=(k == lasts[m]))

            if MODE == "dma_only":
                continue
            if OUT_COMBINE:
                osb_row = spool.tile([P, MT, H], odt, tag="osb", name=f"osb_{r}")
                for m in range(MT):
                    # drain on ACT so the DVE FIFO (one-hots) never queues
                    # behind a drain that waits on the whole row's matmuls
                    nc.scalar.activation(osb_row[:, m, :], psum[m][:],
                                         mybir.ActivationFunctionType.Copy,
                                         scale=invc_sb[:, m:m + 1])
                if MODE != "no_out":
                    orv = out[r, :, :].rearrange("(m p) h -> p m h", p=P)
                    out_eng.dma_start(out=orv[:], in_=osb_row[:])
            else:
                for m in range(MT):
                    osb = spool.tile([P, H], odt, tag="osb", name=f"osb_{r}_{m}")
                    nc.scalar.activation(osb[:], psum[m][:],
                                         mybir.ActivationFunctionType.Copy,
                                         scale=invc_sb[:, m:m + 1])
                    if MODE != "no_out":
                        out_eng.dma_start(out=out[r, m * P:(m + 1) * P, :],
                                          in_=osb[:])
    return body


def _prep(hidden_states, output_mask, precision=None):
    precision = precision or PRECISION
    np_ddt = np.float16 if precision == "fp16" else np.float32
    hs = np.ascontiguousarray(np.asarray(hidden_states).astype(np_ddt))
    assert hs.shape == (B, S, H), hs.shape
    mask = np.asarray(output_mask).astype(np.int64)
    assert mask.shape == (B, S), mask.shape

    valid = mask >= 0
    # per-(row, K-tile) id range over valid tokens
    m3 = mask.reshape(B, KT, P)
    v3 = valid.reshape(B, KT, P)
    lo = np.where(v3, m3, np.iinfo(np.int64).max).min(axis=2)  # [B, KT]
    hi = np.where(v3, m3, -1).max(axis=2)                      # [B, KT]

    klists = []
    for r in range(RPC):
        rows = [c * RPC + r for c in range(NCORES)]
        per_m = []
        for m in range(MT):
            ks = [k for k in range(KT)
                  if any(lo[b, k] <= m * P + P - 1 and hi[b, k] >= m * P
                         for b in rows)]
            per_m.append(ks if ks else [0])
        klists.append(per_m)

    counts = np.zeros((B, NSEG), np.int64)
    for b in range(B):
        ids = mask[b][valid[b]]
        ids = ids[ids < NSEG]
        counts[b] = np.bincount(ids, minlength=NSEG)
    invc = (1.0 / np.maximum(counts, 1)).astype(np.float32)

    maskp = np.ascontiguousarray(
        mask.astype(np.float32).reshape(B, KT, P).transpose(0, 2, 1))
    invcp = np.ascontiguousarray(
        invc.reshape(B, MT, P).transpose(0, 2, 1))

    in_maps = [{
        "x": hs[c * RPC:(c + 1) * RPC],
        "maskp": maskp[c * RPC:(c + 1) * RPC],
        "invc": invcp[c * RPC:(c + 1) * RPC],
    } for c in range(NCORES)]
    return klists, in_maps


_PROGRAM_CACHE = {}


def _get_program(klists):
    key = (PRECISION,
           tuple(tuple(tuple(ks) for ks in per_m) for per_m in klists))
    if key not in _PROGRAM_CACHE:
        _PROGRAM_CACHE[key] = _build_program(klists)
    return _PROGRAM_CACHE[key]


def kernel(hidden_states, output_mask):
    klists, in_maps = _prep(hidden_states, output_mask)
    nc = _get_program(klists)
    res = run_bass_kernel_spmd(nc, in_maps, core_ids=list(range(NCORES)))
    full = np.concatenate(
        [res.results[c]["out"].reshape(RPC * NSEG, H).astype(np.float32)
         for c in range(NCORES)],
        axis=0)
    return full


if __name__ == "__main__":
    rng = np.random.default_rng(0)
    hs = rng.standard_normal((B, S, H)).astype(np.float32)
    mask = np.sort(rng.integers(0, NSEG, size=(B, S)), axis=-1).astype(np.int32)
    out = kernel(hidden_states=hs, output_mask=mask)
    print(out.shape, out.dtype)


# revision 21
# speedup vs baseline: 1.6544x; 1.0218x over previous
"""Segment mean-pool (ContextEncoder) Bass kernel for Trainium2, 8 NeuronCores.

Problem: hidden_states [16, 4096, 1024] f32, output_mask [16, 4096] int
(non-decreasing segment ids per row in [0, 512), -1 = dropped token).
Output [16*512, 1024] f32: mean of tokens sharing (batch, segment id),
zeros for empty segments.

Strategy: data-parallel over batch, 2 rows per core. Per 128-token K-tile,
build a one-hot [tokens x segments] matrix on the vector engine (iota vs
per-partition segment id, is_equal), then accumulate one_hot.T @ x on the
tensor engine (fp16 operands, fp32 PSUM accumulate), one [128 seg x 1024]
PSUM region per 128-segment M-tile. Because ids are sorted, each K-tile
only touches 1-2 M-tiles; the (k -> M-tiles) map is computed on the host
from the actual masks (the program is compiled per input batch) so the
matmul count stays near the minimum while remaining exact for any mask
content. Mean = PSUM * (1/count) on an ACT-engine drain (counts from a
host bincount), written back as fp16 and upcast on the host.

Memory-system notes (measured on HW via in-NEFF loop + paired timing):
input reads stream at ~440 GB/s/core when grouped 2 K-tiles per DMA on a
dedicated HWDGE ring; HBM writes only reach ~100-140 GB/s and barely
overlap reads, so outputs are fp16, coalesced into two half-row DMAs
(the first issues mid-row after its two drains complete).
"""

import numpy as np

import concourse.bass as bass  # noqa: F401  (registers bass_rust)
import concourse.mybir as mybir
import concourse.tile as tile
from concourse import bacc
from concourse.bass_utils import run_bass_kernel_spmd

B, S, H = 16, 4096, 1024
NSEG = 512
NCORES = 8
RPC = B // NCORES          # rows (batch examples) per core
P = 128                    # partitions
KT = S // P                # 32 K-tiles of 128 tokens
MT = NSEG // P             # 4 M-tiles of 128 segments
NH = H // 512              # matmul free-dim chunks (PSUM bank = 512 f32)

F32 = mybir.dt.float32
F32R = mybir.dt.float32r   # full-rate fp32 matmul mode on TRN2
F16 = mybir.dt.float16

# "fp16": half input DMA traffic (memory-bound win), ~2-4e-4 rel err.
# "fp32r": full fp32 traffic, ~1.6e-4 rel err.
PRECISION = "fp16"

# Number of SBUF buffers for data tiles (DMA prefetch depth)
DATA_BUFS = 16
OH_BUFS = 32
OSB_BUFS = 2
KPG = 2            # k-tiles per input DMA (bigger DMAs amortize per-DMA cost)
IN_RING = "sync"   # HWDGE ring for input DMAs (dedicated: avoids head-of-line
OUT_RING = "scalar"  # blocking of inputs behind output DMAs waiting on drains)
MODE = "full"      # "full" | "dma_only" | "compute_only" | "no_out" | "out_only"
OUT_CHUNK = 1      # m-tiles per output DMA (out_only diagnostics)
OUT_ALT = False    # alternate output DMAs across both rings
OUT_FP16 = True    # device writes fp16 outputs (half the slow HBM write
                   # traffic); host upcasts to f32 after gather
OUT_COMBINE = True  # one output DMA per row instead of one per m-tile
OUT_SPLIT = 2       # with OUT_COMBINE: split the row write into this many DMAs
                    # (2 lets the first half issue after only 2 drains)


def _build_program(klists, loop_n=1, precision=None):
    """klists[r][m] -> sorted list of K-tile indices whose token ids (in any
    row assigned to program slot r) overlap segment M-tile m. Must be
    non-empty for every (r, m).

    loop_n > 1 wraps the body in an in-NEFF repeat loop (timing only)."""
    precision = precision or PRECISION
    ddt = F16 if precision == "fp16" else F32R
    nc = bacc.Bacc("TRN2", target_bir_lowering=False, debug=False)
    x = nc.dram_tensor("x", [RPC, S, H], ddt, kind="ExternalInput")
    maskp = nc.dram_tensor("maskp", [RPC, P, KT], F32, kind="ExternalInput")
    invc = nc.dram_tensor("invc", [RPC, P, MT], F32, kind="ExternalInput")
    odt = F16 if OUT_FP16 else F32
    out = nc.dram_tensor("out", [RPC, NSEG, H], odt, kind="ExternalOutput")

    with tile.TileContext(nc) as tc:
        with tc.tile_pool(name="const", bufs=1) as cpool, \
             tc.tile_pool(name="data", bufs=DATA_BUFS) as dpool, \
             tc.tile_pool(name="oh", bufs=OH_BUFS) as opool, \
             tc.tile_pool(name="osb", bufs=OSB_BUFS) as spool, \
             tc.tile_pool(name="ps", bufs=MT, space="PSUM") as pspool:
            iota_t = cpool.tile([P, NSEG], F32, tag="iota")
            nc.gpsimd.iota(iota_t[:], [[1, NSEG]], channel_multiplier=0,
                           allow_small_or_imprecise_dtypes=True)
            body = _make_body(nc, klists, x, maskp, invc, out, iota_t,
                              cpool, dpool, opool, spool, pspool, ddt, odt)
            if loop_n > 1:
                with tc.For_i(0, loop_n, 1):
                    body()
            else:
                body()
    nc.compile()
    return nc


def _make_body(nc, klists, x, maskp, invc, out, iota_t,
               cpool, dpool, opool, spool, pspool, ddt, odt):
    in_eng = getattr(nc, IN_RING)
    out_eng = getattr(nc, OUT_RING)

    def body():
        for r in range(RPC):
            mask_sb = cpool.tile([P, KT], F32, tag=f"mask{r}")
            nc.sync.dma_start(out=mask_sb[:], in_=maskp[r])
            invc_sb = cpool.tile([P, MT], F32, tag=f"invc{r}")
            nc.sync.dma_start(out=invc_sb[:], in_=invc[r])

            k_to_ms = {}
            for m in range(MT):
                for k in klists[r][m]:
                    k_to_ms.setdefault(k, []).append(m)
            firsts = {m: klists[r][m][0] for m in range(MT)}
            lasts = {m: klists[r][m][-1] for m in range(MT)}

            psum = [pspool.tile([P, H], F32, tag="ps", name=f"psum_r{r}m{m}")
                    for m in range(MT)]

            if MODE == "out_only":
                # OUT_CHUNK m-tiles per write DMA; alternate rings if OUT_ALT
                osb0 = spool.tile([P, MT, H], odt, tag="osb", name=f"osb_{r}")
                nc.vector.memset(osb0[:], 0.25)
                orv = out[r, :, :].rearrange("(m p) h -> p m h", p=P)
                for i, m in enumerate(range(0, MT, OUT_CHUNK)):
                    eng = (in_eng if (OUT_ALT and i % 2) else out_eng)
                    eng.dma_start(out=orv[:, m:m + OUT_CHUNK, :],
                                  in_=osb0[:, m:m + OUT_CHUNK, :])
                continue
            # x[r] tokens t = k*P + p; view as [p, k, h] for grouped loads
            xr = x[r, :, :].rearrange("(k p) h -> p k h", p=P)
            cdata = None
            for k0 in range(0, KT, KPG):
                group = [k for k in range(k0, min(k0 + KPG, KT)) if k in k_to_ms]
                if not group:
                    continue
                g = min(k0 + KPG, KT) - k0
                if MODE == "compute_only":
                    if cdata is None:
                        cdata = dpool.tile([P, KPG, H], ddt, tag="data",
                                           name=f"data_{r}")
                        in_eng.dma_start(out=cdata[:], in_=xr[:, 0:KPG, :])
                    data_t = cdata
                else:
                    data_t = dpool.tile([P, g, H], ddt, tag="data",
                                        name=f"data_{r}_{k0}")
                    in_eng.dma_start(out=data_t[:], in_=xr[:, k0:k0 + g, :])
                if MODE == "dma_only":
                    continue
                for k in group:
                    ms = k_to_ms[k]
                    m0, span = ms[0], ms[-1] - ms[0] + 1
                    oh = opool.tile([P, span * P], ddt, tag="oh",
                                    name=f"oh_{r}_{k}")
                    nc.vector.tensor_scalar(
                        out=oh[:], in0=iota_t[:, m0 * P:(m0 + span) * P],
                        scalar1=mask_sb[:, k:k + 1],
                        scalar2=None, op0=mybir.AluOpType.is_equal)
                    for m in ms:
                        for n in range(NH):
                            nc.tensor.matmul(
                                out=psum[m][:, n * 512:(n + 1) * 512],
                                lhsT=oh[:, (m - m0) * P:(m - m0 + 1) * P],
                                rhs=data_t[:, k - k0, n * 512:(n + 1) * 512],
                                start=(k == firsts[m]), stop=(k == lasts[m]))

            if MODE == "dma_only":
                continue
            if OUT_COMBINE:
                osb_row = spool.tile([P, MT, H], odt, tag="osb", name=f"osb_{r}")
                orv = out[r, :, :].rearrange("(m p) h -> p m h", p=P)
                step = MT // OUT_SPLIT
                for m in range(MT):
                    # drain on ACT so the DVE FIFO (one-hots) never queues
                    # behind a drain that waits on the whole row's matmuls
                    nc.scalar.activation(osb_row[:, m, :], psum[m][:],
                                         mybir.ActivationFunctionType.Copy,
                                         scale=invc_sb[:, m:m + 1])
                    if MODE != "no_out" and (m + 1) % step == 0:
                        out_eng.dma_start(out=orv[:, m + 1 - step:m + 1, :],
                                          in_=osb_row[:, m + 1 - step:m + 1, :])
            else:
                for m in range(MT):
                    osb = spool.tile([P, H], odt, tag="osb", name=f"osb_{r}_{m}")
                    nc.scalar.activation(osb[:], psum[m][:],
                                         mybir.ActivationFunctionType.Copy,
                                         scale=invc_sb[:, m:m + 1])
                    if MODE != "no_out":
                        out_eng.dma_start(out=out[r, m * P:(m + 1) * P, :],
                                          in_=osb[:])
    return body


def _prep(hidden_states, output_mask, precision=None):
    precision = precision or PRECISION
    np_ddt = np.float16 if precision == "fp16" else np.float32
    hs = np.ascontiguousarray(np.asarray(hidden_states).astype(np_ddt))
    assert hs.shape == (B, S, H), hs.shape
    mask = np.asarray(output_mask).astype(np.int64)
    assert mask.shape == (B, S), mask.shape

    valid = mask >= 0
    # per-(row, K-tile) id range over valid tokens
    m3 = mask.reshape(B, KT, P)
    v3 = valid.reshape(B, KT, P)
    lo = np.where(v3, m3, np.iinfo(np.int64).max).min(axis=2)  # [B, KT]
    hi = np.where(v3, m3, -1).max(axis=2)                      # [B, KT]

    klists = []
    for r in range(RPC):
        rows = [c * RPC + r for c in range(NCORES)]
        per_m = []
        for m in range(MT):
            ks = [k for k in range(KT)
                  if any(lo[b, k] <= m * P + P - 1 and hi[b, k] >= m * P
                         for b in rows)]
            per_m.append(ks if ks else [0])
        klists.append(per_m)

    counts = np.zeros((B, NSEG), np.int64)
    for b in range(B):
        ids = mask[b][valid[b]]
        ids = ids[ids < NSEG]
        counts[b] = np.bincount(ids, minlength=NSEG)
    invc = (1.0 / np.maximum(counts, 1)).astype(np.float32)

    maskp = np.ascontiguousarray(
        mask.astype(np.float32).reshape(B, KT, P).transpose(0, 2, 1))
    invcp = np.ascontiguousarray(
        invc.reshape(B, MT, P).transpose(0, 2, 1))

    in_maps = [{
        "x": hs[c * RPC:(c + 1) * RPC],
        "maskp": maskp[c * RPC:(c + 1) * RPC],
        "invc": invcp[c * RPC:(c + 1) * RPC],
    } for c in range(NCORES)]
    return klists, in_maps


_PROGRAM_CACHE = {}


def _get_program(klists):
    key = (PRECISION,
           tuple(tuple(tuple(ks) for ks in per_m) for per_m in klists))
    if key not in _PROGRAM_CACHE:
        _PROGRAM_CACHE[key] = _build_program(klists)
    return _PROGRAM_CACHE[key]


def kernel(hidden_states, output_mask):
    klists, in_maps = _prep(hidden_states, output_mask)
    nc = _get_program(klists)
    res = run_bass_kernel_spmd(nc, in_maps, core_ids=list(range(NCORES)))
    full = np.concatenate(
        [res.results[c]["out"].reshape(RPC * NSEG, H).astype(np.float32)
         for c in range(NCORES)],
        axis=0)
    return full


if __name__ == "__main__":
    rng = np.random.default_rng(0)
    hs = rng.standard_normal((B, S, H)).astype(np.float32)
    mask = np.sort(rng.integers(0, NSEG, size=(B, S)), axis=-1).astype(np.int32)
    out = kernel(hidden_states=hs, output_mask=mask)
    print(out.shape, out.dtype)
